# revision 1
# baseline (speedup 1.0000x reference)
"""Trainium2 Bass kernel for nn_DSP_33131377721365 (v2).

reference math (x: [4, 32, 720, 720] f32, conv_w: [32, 32, 3, 1] f32):
  s[b,h,w]    = sum_c x[b,c,h,w]
  d[b,h,w]    = (1/9) * sum_{t=0..8} s[b, h+t-4, w+t-4]   (zero padded)
  out[b,o,h,w]= sum_{j=0..2} wsum[o,j] * d[b, h-1+j, w]   (zero padded)
  where wsum[o,j] = sum_c conv_w[o,c,j,0]

Sharding: 8 cores = 4 batches x 2 H-halves (360 rows each); host pre-pads each
shard with 5 halo rows. All HBM I/O in bf16 (host casts) - halves the memory
roofline vs f32.

Per core, 4 H-blocks of 90 output rows, rows on SBUF partitions:
 1. channel sum s: 5 rounds of contiguous-halves tensor_tensor adds on DVE
    (2x bf16 mode; tensor_reduce would be 1x and ~2.1x slower).
 2. 9-tap diagonal pool: 9 accumulating PE matmuls into one PSUM bank; tap t
    uses a shifted-identity band lhsT (h-shift) and a free-dim offset t on the
    rhs AP (w-shift). No SBUF->SBUF shift copies at all.
 3. d evacuated (ScalarE, f32->bf16 cast) into three window tiles for the
    3x1 conv; sub-blocks are (32, 32, 26) output rows so every engine AP has
    a 32-aligned base partition (HW requirement: base % 32 == 0, any size).
 4. 3x1 conv + broadcast to 32 output channels: banded matmuls with 4 output
    channels x 32 rows packed into M=128 (104 for the tail sub-block).
    PSUM evacuated by ScalarE/VectorE (split) with bf16 cast, DMA out on the
    ACT HWDGE ring (input DMAs ride the SP ring).
Host reassembles/casts the bf16 output back to f32.
"""

import numpy as np
import ml_dtypes

import concourse.bass as bass
import concourse.bacc as bacc
import concourse.mybir as mybir
import concourse.tile as tile
from concourse.bass_utils import run_bass_kernel_spmd

FP = mybir.dt.float32
BF = mybir.dt.bfloat16
NPBF = ml_dtypes.bfloat16

B, C, H, W = 4, 32, 720, 720
O = 32
N_CORES = 8
HS = H // 2          # 360 output rows per core
BLK = 90             # output rows per block
NBLK = HS // BLK     # 4
SROWS = BLK + 10     # 100 s-rows per block (pool 8 + conv 2 halo)
M1 = BLK + 2         # 92 d rows per block
SUBNS = (32, 32, 26)  # stage-2 output rows per sub-block (32-aligned windows)
OSUB = 4             # output channels per stage-2 matmul
NOG = O // OSUB      # 8 o-groups
M2MAX = 128
WPAD = 4
SPW = W + 2 * WPAD   # 728
KTAPS = 9
HALO = 5

# stage-2 evacuation engine split: og-pair indices handled by DVE (rest ACT)
DVE_EVAC_OPS = (2, 3)
# stage-1 window evac engine: "scalar" or "vector"
S1_EVAC = "scalar"


def _build(nc, reps=1):
    xs = nc.declare_dram_parameter("xs", [HS + 2 * HALO, C, W], BF, isOutput=False)
    bd = nc.declare_dram_parameter("bands", [SROWS, KTAPS, NBLK, M1], BF,
                                   isOutput=False)
    ama = nc.declare_dram_parameter("amca", [34, NOG, 128], BF, isOutput=False)
    amb = nc.declare_dram_parameter("amcb", [28, NOG, 104], BF, isOutput=False)
    # [blk, p, s, og, w]: one DMA per block, 3*8*1440B contiguous/partition
    out = nc.declare_dram_parameter("out", [NBLK, M2MAX, 3, NOG, W], BF,
                                    isOutput=True)

    add = mybir.AluOpType.add

    with tile.TileContext(nc) as tc:
        with (
            tc.tile_pool(name="xa", bufs=2) as xpool,
            tc.tile_pool(name="tr", bufs=1) as tpool,
            tc.tile_pool(name="sp", bufs=2) as spool,
            tc.tile_pool(name="dd", bufs=2) as dpool,
            tc.tile_pool(name="ob", bufs=1) as opool,
            tc.tile_pool(name="cst", bufs=1) as cpool,
            tc.tile_pool(name="ps1", bufs=2, space="PSUM") as ps1pool,
            tc.tile_pool(name="ps2", bufs=2, space="PSUM") as ps2pool,
        ):
            bdt = cpool.tile([SROWS, KTAPS, NBLK, M1], BF)
            nc.sync.dma_start(bdt[:], bd[:])
            amta = cpool.tile([34, NOG, 128], BF)
            nc.sync.dma_start(amta[:], ama[:])
            amtb = cpool.tile([28, NOG, 104], BF)
            nc.sync.dma_start(amtb[:], amb[:])

            for it in range(NBLK * reps):
                blk = it % NBLK
                r0 = blk * BLK

                xa = xpool.tile([SROWS, C, W], BF, tag="xa")
                nc.sync.dma_start(xa[:], xs[r0:r0 + SROWS])

                # channel sum via contiguous-halves tree (bf16 2x DVE mode),
                # split by W-halves so stage-1 chunk 0 can start early.
                # t3/t4 alias dead regions of t1/t2 to save SBUF.
                t1 = tpool.tile([SROWS, 16, W], BF, tag="t1")
                t2 = tpool.tile([SROWS, 8, W], BF, tag="t2")
                sp = spool.tile([SROWS, SPW], BF, tag="sp")
                nc.vector.memset(sp[:, 0:WPAD], 0.0)
                nc.vector.memset(sp[:, WPAD + W:SPW], 0.0)
                for h0 in (0, 360):
                    h1 = h0 + 360
                    nc.vector.tensor_tensor(out=t1[:, :, h0:h1],
                                            in0=xa[:, 0:16, h0:h1],
                                            in1=xa[:, 16:32, h0:h1], op=add)
                    nc.vector.tensor_tensor(out=t2[:, :, h0:h1],
                                            in0=t1[:, 0:8, h0:h1],
                                            in1=t1[:, 8:16, h0:h1], op=add)
                    t3 = t1[:, 0:4, :]
                    nc.vector.tensor_tensor(out=t3[:, :, h0:h1],
                                            in0=t2[:, 0:4, h0:h1],
                                            in1=t2[:, 4:8, h0:h1], op=add)
                    t4 = t2[:, 0:2, :]
                    nc.vector.tensor_tensor(out=t4[:, :, h0:h1],
                                            in0=t3[:, 0:2, h0:h1],
                                            in1=t3[:, 2:4, h0:h1], op=add)
                    nc.vector.tensor_tensor(out=sp[:, WPAD + h0:WPAD + h1],
                                            in0=t4[:, 0, h0:h1],
                                            in1=t4[:, 1, h0:h1], op=add)

                # 9-tap diagonal pool on PE; evacuate the three conv windows
                # (psum rows [0:34),[32:66),[64:92) - all 32-aligned bases)
                ds0 = dpool.tile([34, W], BF, tag="ds0")
                ds1 = dpool.tile([34, W], BF, tag="ds1")
                ds2 = dpool.tile([28, W], BF, tag="ds2")
                dwin = (ds0, ds1, ds2)
                for w0 in (0, 360):
                    ps1 = ps1pool.tile([M1, 360], FP, tag="ps1")
                    for t in range(KTAPS):
                        nc.tensor.matmul(
                            ps1[:], bdt[:, t, blk, :],
                            sp[:, w0 + t:w0 + t + 360],
                            start=(t == 0), stop=(t == KTAPS - 1),
                        )
                    # engine APs: any size from partition 0, else <=32 parts
                    e1 = nc.scalar if S1_EVAC == "scalar" else nc.vector
                    cp1 = e1.copy if S1_EVAC == "scalar" else e1.tensor_copy
                    cp1(out=ds0[0:34, w0:w0 + 360], in_=ps1[0:34, :])
                    cp1(out=ds1[0:32, w0:w0 + 360], in_=ps1[32:64, :])
                    cp1(out=ds1[32:34, w0:w0 + 360], in_=ps1[64:66, :])
                    cp1(out=ds2[0:28, w0:w0 + 360], in_=ps1[64:92, :])

                # 3x1 conv + channel broadcast: banded matmuls, M=4o x ns
                ob3 = opool.tile([M2MAX, 3, NOG, W], BF, tag="ob")
                for s in range(3):
                    ns = SUBNS[s]
                    kw, m2 = ns + 2, OSUB * ns
                    amt = amta if s < 2 else amtb
                    dsl = dwin[s]
                    for op in range(NOG // 2):   # og pairs share a psum tile
                        # [128, 1440] f32 = 3 banks; each matmul chunk within
                        # a 512-f32 bank: A@[0,512), A@[512,720), B@[720,1024),
                        # B@[1024,1440). Pair is contiguous -> one evac copy.
                        ps2 = ps2pool.tile([M2MAX, 2 * W], FP, tag="ps2")
                        lhsA = amt[:, 2 * op, :]
                        lhsB = amt[:, 2 * op + 1, :]
                        nc.tensor.matmul(ps2[0:m2, 0:512], lhsA,
                                         dsl[0:kw, 0:512],
                                         start=True, stop=True)
                        nc.tensor.matmul(ps2[0:m2, 512:720], lhsA,
                                         dsl[0:kw, 512:W],
                                         start=True, stop=True)
                        nc.tensor.matmul(ps2[0:m2, 720:1024], lhsB,
                                         dsl[0:kw, 0:304],
                                         start=True, stop=True)
                        nc.tensor.matmul(ps2[0:m2, 1024:1440], lhsB,
                                         dsl[0:kw, 304:W],
                                         start=True, stop=True)
                        dst = ob3[0:m2, s, 2 * op:2 * op + 2, :]
                        src = ps2[0:m2, :].rearrange("p (g w) -> p g w", g=2)
                        if op in DVE_EVAC_OPS:
                            nc.vector.tensor_copy(out=dst, in_=src)
                        else:
                            nc.scalar.copy(out=dst, in_=src)
                nc.scalar.dma_start(out[blk], ob3[:])
    return nc


def _make_bands(half):
    """[SROWS, 9, NBLK, M1] bf16: tap-t shifted-identity bands, 1/9 scaled.
    Zero the d rows that fall outside the global image (conv zero padding)."""
    bands = np.zeros((SROWS, KTAPS, NBLK, M1), np.float32)
    for t in range(KTAPS):
        for m in range(M1):
            k = m + t
            if k < SROWS:
                bands[k, t, :, m] = 1.0 / KTAPS
    if half == 0:
        bands[:, :, 0, 0] = 0.0        # d row h=-1
    else:
        bands[:, :, NBLK - 1, M1 - 1] = 0.0  # d row h=720
    return bands.astype(NPBF)


def _make_amc(conv_w, ns):
    """[ns+2, NOG, 4*ns] bf16: 3-tap conv bands, 4 o x ns h packed."""
    wsum = conv_w.sum(axis=1)[:, :, 0].astype(np.float64)  # [O, 3]
    amc = np.zeros((ns + 2, NOG, OSUB * ns), np.float32)
    for og in range(NOG):
        for oi in range(OSUB):
            o = og * OSUB + oi
            for m in range(ns):
                for j in range(3):
                    amc[m + j, og, oi * ns + m] = wsum[o, j]
    return amc.astype(NPBF)


def _make_shard(xt_b, h0):
    """xt_b: [H, C, W] bf16 one batch (h-major). Returns padded [HS+10, C, W]."""
    sh = np.zeros((HS + 2 * HALO, C, W), NPBF)
    lo, hi = h0 - HALO, h0 + HS + HALO
    slo, shi = max(lo, 0), min(hi, H)
    sh[slo - lo:shi - lo] = xt_b[slo:shi]
    return sh


def make_in_maps(x, conv_w):
    x = np.ascontiguousarray(np.asarray(x, dtype=np.float32))
    conv_w = np.asarray(conv_w, dtype=np.float32)
    assert x.shape == (B, C, H, W) and conv_w.shape == (O, C, 3, 1)
    xt = np.ascontiguousarray(x.transpose(0, 2, 1, 3)).astype(NPBF)  # [B,H,C,W]
    amca = _make_amc(conv_w, 32)
    amcb = _make_amc(conv_w, 26)
    bands = [_make_bands(0), _make_bands(1)]
    in_maps = []
    for i in range(N_CORES):
        b, half = i // 2, i % 2
        in_maps.append({
            "xs": _make_shard(xt[b], half * HS),
            "bands": bands[half],
            "amca": amca,
            "amcb": amcb,
        })
    return in_maps


def assemble_out(results):
    out = np.empty((B, O, H, W), np.float32)
    for i in range(N_CORES):
        b, half = i // 2, i % 2
        v = np.asarray(results[i]["out"]).astype(np.float32)  # [4,128,3,NOG,W]
        ov = np.empty((O, HS, W), np.float32)
        for blk in range(NBLK):
            for s in range(3):
                ns = SUBNS[s]
                h0 = blk * BLK + 32 * s
                w = v[blk, 0:OSUB * ns, s, :, :]        # [4*ns, NOG, W]
                w = w.reshape(OSUB, ns, NOG, W)
                w = w.transpose(2, 0, 1, 3).reshape(O, ns, W)
                ov[:, h0:h0 + ns, :] = w
        out[b, :, half * HS:(half + 1) * HS, :] = ov
    return out


def kernel(x, conv_w):
    nc = bacc.Bacc("TRN2", target_bir_lowering=False, debug=False,
                   num_devices=N_CORES)
    _build(nc)
    nc.compile()
    res = run_bass_kernel_spmd(nc, make_in_maps(x, conv_w),
                               list(range(N_CORES)), trace=False)
    return assemble_out(res.results)



# revision 2
# speedup vs baseline: 1.2863x; 1.2863x over previous
"""Trainium2 Bass kernel for nn_DSP_33131377721365 (v3).

reference math (x: [4, 32, 720, 720] f32, conv_w: [32, 32, 3, 1] f32):
  s[b,h,w]    = sum_c x[b,c,h,w]
  d[b,h,w]    = (1/9) * sum_{t=0..8} s[b, h+t-4, w+t-4]   (zero padded)
  out[b,o,h,w]= sum_{j=0..2} wsum[o,j] * d[b, h-1+j, w]   (zero padded)
  where wsum[o,j] = sum_c conv_w[o,c,j,0]

Sharding: 8 cores = 4 batches x 2 H-halves (360 output rows each). All HBM
I/O bf16 (host casts).

v3 changes vs v2 (both measured on the axon-tunneled cores):
 * Input rides 3 disjoint [128, 23040] row-aligned DMAs (shard zero-padded
   to 384 rows). 128-partition transfers run ~3x faster than the v2
   [100, C, W] tiles (616 vs 191 GB/s measured): partition count must be
   128 to engage all 16 SDMA engines evenly.
 * Channel-sum tree runs per 128-row chunk (full partitions) in W-halves.
 * Stage-1 9-tap diagonal pool: per (block, tap) the 100-row s-window may
   straddle two chunks; bands are [128, 92] per (block, tap, chunk) and
   the matmuls accumulate across chunk parts in PSUM. K=128 from partition
   0 dodges the engine-AP base%32 rule.
 * Stage-2 packs output rows STRIDED by 3: sub-block s = out rows
   {s, s+3, ..., s+87} (30 rows x 4 channels = M 120). Every sub-block then
   reads d[0:92] from partition 0, so stage-1 PSUM evacuates with ONE copy
   per W-half into a single [92, 720] d tile (v2 needed 4 copies into 3
   window tiles), and all stage-2 rhs APs are legal.
Host reassembles/casts the bf16 output back to f32 (un-striding rows).
"""

import numpy as np
import ml_dtypes

import concourse.bass as bass
import concourse.bacc as bacc
import concourse.mybir as mybir
import concourse.tile as tile
from concourse.bass_utils import run_bass_kernel_spmd

FP = mybir.dt.float32
BF = mybir.dt.bfloat16
NPBF = ml_dtypes.bfloat16

B, C, H, W = 4, 32, 720, 720
O = 32
N_CORES = 8
HS = H // 2          # 360 output rows per core
BLK = 90             # output rows per block
NBLK = HS // BLK     # 4
SHR = 384            # shard rows (370 used + 14 zero pad), 3 chunks of 128
NCH = 3              # chunks per core
ROW = C * W          # 23040 elems per shard row
M1 = BLK + 2         # 92 d rows per block
NS = 30              # rows per stage-2 sub-block (strided by 3)
OSUB = 4             # output channels per stage-2 matmul group
NOG = O // OSUB      # 8 o-groups
M2 = OSUB * NS       # 120
WPAD = 4
SPW = W + 2 * WPAD   # 728
KTAPS = 9
HALO = 5

# stage-2 evacuation engine split: og-pair indices handled by DVE (rest ACT)
DVE_EVAC_OPS = (2, 3)


def _mm_parts(b):
    """[(tap, chunk, rows_in_chunk...)]: which chunks tap t of block b hits."""
    parts = []
    for t in range(KTAPS):
        lo, hi = 90 * b + t, 90 * b + t + M1  # shard rows [lo, hi)
        for c in range(NCH):
            clo, chi = 128 * c, 128 * (c + 1)
            if lo < chi and hi > clo:
                parts.append((t, c))
    return parts


def _build(nc, reps=1):
    xs = nc.declare_dram_parameter("xs", [SHR, ROW], BF, isOutput=False)
    nmm = sum(len(_mm_parts(b)) for b in range(NBLK))
    bd = nc.declare_dram_parameter("bands", [128, nmm, M1], BF, isOutput=False)
    am = nc.declare_dram_parameter("amc", [M1, 3, NOG, M2], BF, isOutput=False)
    # [blk, p, s, og, w]: one DMA per block; rows 120..128 junk
    out = nc.declare_dram_parameter("out", [NBLK, 128, 3, NOG, W], BF,
                                    isOutput=True)

    add = mybir.AluOpType.add

    with tile.TileContext(nc) as tc:
        with (
            tc.tile_pool(name="xa", bufs=2) as xpool,
            tc.tile_pool(name="tr", bufs=1) as tpool,
            tc.tile_pool(name="sp", bufs=3) as spool,
            tc.tile_pool(name="dd", bufs=2) as dpool,
            tc.tile_pool(name="ob", bufs=2) as opool,
            tc.tile_pool(name="cst", bufs=1) as cpool,
            tc.tile_pool(name="ps1", bufs=2, space="PSUM") as ps1pool,
            tc.tile_pool(name="ps2", bufs=2, space="PSUM") as ps2pool,
        ):
            bdt = cpool.tile([128, nmm, M1], BF)
            nc.sync.dma_start(bdt[:], bd[:])
            amt = cpool.tile([M1, 3, NOG, M2], BF)
            nc.sync.dma_start(amt[:], am[:])

            for it in range(reps):
                sps = []
                for ci in range(NCH):
                    xa = xpool.tile([128, ROW], BF, tag="xa", name=f"xa{ci}")
                    nc.sync.dma_start(xa[:], xs[128 * ci:128 * (ci + 1)])
                    xv = xa.rearrange("p (c w) -> p c w", c=C)
                    sp = spool.tile([128, SPW], BF, tag="sp", name=f"sp{ci}")
                    nc.vector.memset(sp[:, 0:WPAD], 0.0)
                    nc.vector.memset(sp[:, WPAD + W:SPW], 0.0)
                    # channel-sum tree in W-halves (bf16 2x DVE mode)
                    t1 = tpool.tile([128, 16, 360], BF, tag="t1", name="t1")
                    t2 = tpool.tile([128, 8, 360], BF, tag="t2", name="t2")
                    for h0 in (0, 360):
                        h1 = h0 + 360
                        nc.vector.tensor_tensor(out=t1[:],
                                                in0=xv[:, 0:16, h0:h1],
                                                in1=xv[:, 16:32, h0:h1],
                                                op=add)
                        nc.vector.tensor_tensor(out=t2[:], in0=t1[:, 0:8],
                                                in1=t1[:, 8:16], op=add)
                        t3 = t1[:, 0:4, :]
                        nc.vector.tensor_tensor(out=t3, in0=t2[:, 0:4],
                                                in1=t2[:, 4:8], op=add)
                        t4 = t2[:, 0:2, :]
                        nc.vector.tensor_tensor(out=t4, in0=t3[:, 0:2],
                                                in1=t3[:, 2:4], op=add)
                        nc.vector.tensor_tensor(
                            out=sp[:, WPAD + h0:WPAD + h1],
                            in0=t4[:, 0], in1=t4[:, 1], op=add)
                    sps.append(sp)

                mi = 0
                for blk in range(NBLK):
                    parts = _mm_parts(blk)
                    # 9-tap diagonal pool: accumulate over (tap, chunk) parts
                    dt = dpool.tile([M1, W], BF, tag="dt", name="dt")
                    for w0 in (0, 360):
                        ps1 = ps1pool.tile([M1, 360], FP, tag="ps1",
                                           name="ps1")
                        for pi, (t, ci) in enumerate(parts):
                            nc.tensor.matmul(
                                ps1[:], bdt[:, mi + pi, :],
                                sps[ci][:, w0 + t:w0 + t + 360],
                                start=(pi == 0), stop=(pi == len(parts) - 1),
                            )
                        nc.scalar.copy(out=dt[:, w0:w0 + 360], in_=ps1[:])
                    mi += len(parts)

                    # 3x1 conv + channel broadcast: strided-row packing,
                    # M = 4 channels x 30 rows; og pairs share a psum tile
                    ob3 = opool.tile([128, 3, NOG, W], BF, tag="ob",
                                     name="ob3")
                    for s in range(3):
                        for op in range(NOG // 2):
                            ps2 = ps2pool.tile([M2, 2 * W], FP, tag="ps2",
                                               name="ps2")
                            lhsA = amt[:, s, 2 * op, :]
                            lhsB = amt[:, s, 2 * op + 1, :]
                            nc.tensor.matmul(ps2[:, 0:512], lhsA,
                                             dt[0:M1, 0:512],
                                             start=True, stop=True)
                            nc.tensor.matmul(ps2[:, 512:720], lhsA,
                                             dt[0:M1, 512:W],
                                             start=True, stop=True)
                            nc.tensor.matmul(ps2[:, 720:1024], lhsB,
                                             dt[0:M1, 0:304],
                                             start=True, stop=True)
                            nc.tensor.matmul(ps2[:, 1024:1440], lhsB,
                                             dt[0:M1, 304:W],
                                             start=True, stop=True)
                            dst = ob3[0:M2, s, 2 * op:2 * op + 2, :]
                            src = ps2[:].rearrange("p (g w) -> p g w", g=2)
                            if op in DVE_EVAC_OPS:
                                nc.vector.tensor_copy(out=dst, in_=src)
                            else:
                                nc.scalar.copy(out=dst, in_=src)
                    nc.scalar.dma_start(out[blk], ob3[:])
    return nc


def _make_bands(half):
    """[128, nmm, 92] bf16 stage-1 bands, one [128, 92] slab per
    (block, tap, chunk) matmul, 1/9 scaled; d rows outside the global image
    zeroed (conv zero padding)."""
    nmm = sum(len(_mm_parts(b)) for b in range(NBLK))
    bands = np.zeros((128, nmm, M1), np.float32)
    mi = 0
    for b in range(NBLK):
        for (t, c) in _mm_parts(b):
            for m in range(M1):
                if half == 0 and b == 0 and m == 0:
                    continue  # d row -1
                if half == 1 and b == NBLK - 1 and m == M1 - 1:
                    continue  # d row 720
                r = 90 * b + m + t - 128 * c
                if 0 <= r < 128:
                    bands[r, mi, m] = 1.0 / KTAPS
            mi += 1
    return bands.astype(NPBF)


def _make_amc(conv_w):
    """[92, 3, NOG, 120] bf16: 3-tap conv bands, strided-row packing.
    Sub-block s, slot oi*30+m -> out channel og*4+oi, block row 3m+s."""
    wsum = conv_w.sum(axis=1)[:, :, 0].astype(np.float64)  # [O, 3]
    amc = np.zeros((M1, 3, NOG, M2), np.float32)
    for s in range(3):
        for og in range(NOG):
            for oi in range(OSUB):
                o = og * OSUB + oi
                for m in range(NS):
                    for j in range(3):
                        amc[3 * m + s + j, s, og, oi * NS + m] = wsum[o, j]
    return amc.astype(NPBF)


def _make_shard(xt_b, h0):
    """xt_b: [H, ROW] bf16 one batch (h-major rows). [384, ROW] zero-pad."""
    sh = np.zeros((SHR, ROW), NPBF)
    lo, hi = h0 - HALO, h0 + HS + HALO
    slo, shi = max(lo, 0), min(hi, H)
    sh[slo - lo:shi - lo] = xt_b[slo:shi]
    return sh


def make_in_maps(x, conv_w):
    x = np.ascontiguousarray(np.asarray(x, dtype=np.float32))
    conv_w = np.asarray(conv_w, dtype=np.float32)
    assert x.shape == (B, C, H, W) and conv_w.shape == (O, C, 3, 1)
    xt = np.ascontiguousarray(x.transpose(0, 2, 1, 3)).astype(NPBF)
    xt = xt.reshape(B, H, ROW)
    amc = _make_amc(conv_w)
    bands = [_make_bands(0), _make_bands(1)]
    in_maps = []
    for i in range(N_CORES):
        b, half = i // 2, i % 2
        in_maps.append({
            "xs": _make_shard(xt[b], half * HS),
            "bands": bands[half],
            "amc": amc,
        })
    return in_maps


def assemble_out(results):
    out = np.empty((B, O, H, W), np.float32)
    for i in range(N_CORES):
        b, half = i // 2, i % 2
        v = np.asarray(results[i]["out"]).astype(np.float32)
        ov = np.empty((O, HS, W), np.float32)
        for blk in range(NBLK):
            for s in range(3):
                w = v[blk, 0:M2, s, :, :]             # [4*30, NOG, W]
                w = w.reshape(OSUB, NS, NOG, W)
                w = w.transpose(2, 0, 1, 3).reshape(O, NS, W)
                ov[:, blk * BLK + s:blk * BLK + 90:3, :] = w
        out[b, :, half * HS:(half + 1) * HS, :] = ov
    return out


def kernel(x, conv_w):
    nc = bacc.Bacc("TRN2", target_bir_lowering=False, debug=False,
                   num_devices=N_CORES)
    _build(nc)
    nc.compile()
    res = run_bass_kernel_spmd(nc, make_in_maps(x, conv_w),
                               list(range(N_CORES)), trace=False)
    return assemble_out(res.results)


# revision 11
# speedup vs baseline: 2.5118x; 1.9528x over previous
"""Trainium2 Bass kernel for nn_DSP_33131377721365 (v3).

reference math (x: [4, 32, 720, 720] f32, conv_w: [32, 32, 3, 1] f32):
  s[b,h,w]    = sum_c x[b,c,h,w]
  d[b,h,w]    = (1/9) * sum_{t=0..8} s[b, h+t-4, w+t-4]   (zero padded)
  out[b,o,h,w]= sum_{j=0..2} wsum[o,j] * d[b, h-1+j, w]   (zero padded)
  where wsum[o,j] = sum_c conv_w[o,c,j,0]

Sharding: 8 cores = 4 batches x 2 H-halves (360 output rows each). All HBM
I/O bf16 (host casts).

v3 changes vs v2 (both measured on the axon-tunneled cores):
 * Input rides 3 disjoint [128, 23040] row-aligned DMAs (shard zero-padded
   to 384 rows). 128-partition transfers run ~3x faster than the v2
   [100, C, W] tiles (616 vs 191 GB/s measured): partition count must be
   128 to engage all 16 SDMA engines evenly.
 * Channel-sum tree runs per 128-row chunk (full partitions) in W-halves.
 * Stage-1 9-tap diagonal pool: per (block, tap) the 100-row s-window may
   straddle two chunks; bands are [128, 92] per (block, tap, chunk) and
   the matmuls accumulate across chunk parts in PSUM. K=128 from partition
   0 dodges the engine-AP base%32 rule.
 * Stage-2 packs output rows STRIDED by 3: sub-block s = out rows
   {s, s+3, ..., s+87} (30 rows x 4 channels = M 120). Every sub-block then
   reads d[0:92] from partition 0, so stage-1 PSUM evacuates with ONE copy
   per W-half into a single [92, 720] d tile (v2 needed 4 copies into 3
   window tiles), and all stage-2 rhs APs are legal.
Host reassembles/casts the bf16 output back to f32 (un-striding rows).
"""

import numpy as np
import ml_dtypes

import concourse.bass as bass
import concourse.bacc as bacc
import concourse.mybir as mybir
import concourse.tile as tile
from concourse.bass_utils import run_bass_kernel_spmd

FP = mybir.dt.float32
BF = mybir.dt.bfloat16
NPBF = ml_dtypes.bfloat16

B, C, H, W = 4, 32, 720, 720
O = 32
N_CORES = 8
HS = H // 2          # 360 output rows per core
BLK = 90             # output rows per block
NBLK = HS // BLK     # 4
SHR = 370            # shard rows; 3 chunks of 128 (last end-aligned)
NCH = 3              # chunks per core
ROW = C * W          # 23040 elems per shard row
M1 = BLK + 2         # 92 d rows per block
NS = 30              # rows per stage-2 sub-block (strided by 3)
OSUB = 4             # output channels per stage-2 matmul group
NOG = O // OSUB      # 8 o-groups
M2 = OSUB * NS       # 120
WPAD = 4
SPW = W + 2 * WPAD   # 728
KTAPS = 9
HALO = 5

# stage-2 evacuation engine split: (s, og-pair) combos handled by DVE (rest
# ACT). Balance: DVE has the channel-sum tree (~37us/rep), ACT the stage-1
# evac; cost-model rates: ACT 1.385us, DVE 1.63us per stage-2 evac op.
# ~11 of 48 ops/rep on DVE equalizes both at ~57us/rep.
DVE_EVAC_OPS = ((0, 3), (1, 3), (2, 3))

# chunk start rows: disjoint except chunk 2, end-aligned to the 370-row
# shard (saves 14 zero-pad rows of HBM traffic). CLAIM = which shard rows
# each chunk OWNS for band construction (chunks 1/2 overlap in rows
# 242..256; chunk 1 owns them).
CH0 = (0, 128, SHR - 128)
CLAIM = ((0, 128), (128, 256), (256, SHR))


def _mm_parts(b):
    """[(tap, chunk)]: which chunks own rows that tap t of block b needs."""
    parts = []
    for t in range(KTAPS):
        lo, hi = 90 * b + t, 90 * b + t + M1  # shard rows [lo, hi)
        for c in range(NCH):
            clo, chi = CLAIM[c]
            if lo < chi and hi > clo:
                parts.append((t, c))
    return parts


def _build(nc, reps=1):
    xs = nc.declare_dram_parameter("xs", [SHR, ROW], BF, isOutput=False)
    nmm = sum(len(_mm_parts(b)) for b in range(NBLK))
    bd = nc.declare_dram_parameter("bands", [128, nmm, M1], BF, isOutput=False)
    am = nc.declare_dram_parameter("amc", [M1, 3, NOG, M2], BF, isOutput=False)
    # [blk, p, s, og, w]: one DMA per block; rows 120..128 junk
    out = nc.declare_dram_parameter("out", [NBLK, 128, 3, NOG, W], BF,
                                    isOutput=True)

    add = mybir.AluOpType.add

    with tile.TileContext(nc) as tc:
        with (
            tc.tile_pool(name="xa", bufs=2) as xpool,
            tc.tile_pool(name="tr", bufs=1) as tpool,
            tc.tile_pool(name="sp", bufs=3) as spool,
            tc.tile_pool(name="dd", bufs=2) as dpool,
            tc.tile_pool(name="ob", bufs=2) as opool,
            tc.tile_pool(name="cst", bufs=1) as cpool,
            tc.tile_pool(name="ps1", bufs=2, space="PSUM") as ps1pool,
            tc.tile_pool(name="ps2", bufs=2, space="PSUM") as ps2pool,
        ):
            bdt = cpool.tile([128, nmm, M1], BF)
            nc.sync.dma_start(bdt[:], bd[:])
            amt = cpool.tile([M1, 3, NOG, M2], BF)
            nc.sync.dma_start(amt[:], am[:])

            for it in range(reps):
                sps = []
                for ci in range(NCH):
                    xa = xpool.tile([128, ROW], BF, tag="xa", name=f"xa{ci}")
                    nc.sync.dma_start(xa[:], xs[CH0[ci]:CH0[ci] + 128])
                    xv = xa.rearrange("p (c w) -> p c w", c=C)
                    sp = spool.tile([128, SPW], BF, tag="sp", name=f"sp{ci}")
                    nc.vector.memset(sp[:, 0:WPAD], 0.0)
                    nc.vector.memset(sp[:, WPAD + W:SPW], 0.0)
                    # channel-sum tree in W-halves (bf16 2x DVE mode)
                    t1 = tpool.tile([128, 16, 360], BF, tag="t1", name="t1")
                    t2 = tpool.tile([128, 8, 360], BF, tag="t2", name="t2")
                    for h0 in (0, 360):
                        h1 = h0 + 360
                        nc.vector.tensor_tensor(out=t1[:],
                                                in0=xv[:, 0:16, h0:h1],
                                                in1=xv[:, 16:32, h0:h1],
                                                op=add)
                        nc.vector.tensor_tensor(out=t2[:], in0=t1[:, 0:8],
                                                in1=t1[:, 8:16], op=add)
                        t3 = t1[:, 0:4, :]
                        nc.vector.tensor_tensor(out=t3, in0=t2[:, 0:4],
                                                in1=t2[:, 4:8], op=add)
                        t4 = t2[:, 0:2, :]
                        nc.vector.tensor_tensor(out=t4, in0=t3[:, 0:2],
                                                in1=t3[:, 2:4], op=add)
                        nc.vector.tensor_tensor(
                            out=sp[:, WPAD + h0:WPAD + h1],
                            in0=t4[:, 0], in1=t4[:, 1], op=add)
                    sps.append(sp)

                mi = 0
                for blk in range(NBLK):
                    parts = _mm_parts(blk)
                    # 9-tap diagonal pool: accumulate over (tap, chunk) parts
                    dt = dpool.tile([M1, W], BF, tag="dt", name="dt")
                    for w0 in (0, 360):
                        ps1 = ps1pool.tile([M1, 360], FP, tag="ps1",
                                           name="ps1")
                        for pi, (t, ci) in enumerate(parts):
                            nc.tensor.matmul(
                                ps1[:], bdt[:, mi + pi, :],
                                sps[ci][:, w0 + t:w0 + t + 360],
                                start=(pi == 0), stop=(pi == len(parts) - 1),
                            )
                        nc.scalar.copy(out=dt[:, w0:w0 + 360], in_=ps1[:])
                    mi += len(parts)

                    # 3x1 conv + channel broadcast: strided-row packing,
                    # M = 4 channels x 30 rows; og pairs share a psum tile
                    ob3 = opool.tile([128, 3, NOG, W], BF, tag="ob",
                                     name="ob3")
                    for s in range(3):
                        for op in range(NOG // 2):
                            ps2 = ps2pool.tile([M2, 2 * W], FP, tag="ps2",
                                               name="ps2")
                            lhsA = amt[:, s, 2 * op, :]
                            lhsB = amt[:, s, 2 * op + 1, :]
                            nc.tensor.matmul(ps2[:, 0:512], lhsA,
                                             dt[0:M1, 0:512],
                                             start=True, stop=True)
                            nc.tensor.matmul(ps2[:, 512:720], lhsA,
                                             dt[0:M1, 512:W],
                                             start=True, stop=True)
                            nc.tensor.matmul(ps2[:, 720:1024], lhsB,
                                             dt[0:M1, 0:304],
                                             start=True, stop=True)
                            nc.tensor.matmul(ps2[:, 1024:1440], lhsB,
                                             dt[0:M1, 304:W],
                                             start=True, stop=True)
                            dst = ob3[0:M2, s, 2 * op:2 * op + 2, :]
                            src = ps2[:].rearrange("p (g w) -> p g w", g=2)
                            if (s, op) in DVE_EVAC_OPS:
                                nc.vector.tensor_copy(out=dst, in_=src)
                            else:
                                nc.scalar.copy(out=dst, in_=src)
                    nc.scalar.dma_start(out[blk], ob3[:])
    return nc


def _make_bands(half):
    """[128, nmm, 92] bf16 stage-1 bands, one [128, 92] slab per
    (block, tap, chunk) matmul, 1/9 scaled; d rows outside the global image
    zeroed (conv zero padding)."""
    nmm = sum(len(_mm_parts(b)) for b in range(NBLK))
    bands = np.zeros((128, nmm, M1), np.float32)
    mi = 0
    for b in range(NBLK):
        for (t, c) in _mm_parts(b):
            clo, chi = CLAIM[c]
            for m in range(M1):
                if half == 0 and b == 0 and m == 0:
                    continue  # d row -1
                if half == 1 and b == NBLK - 1 and m == M1 - 1:
                    continue  # d row 720
                r = 90 * b + m + t
                if clo <= r < chi:
                    bands[r - CH0[c], mi, m] = 1.0 / KTAPS
            mi += 1
    return bands.astype(NPBF)


def _make_amc(conv_w):
    """[92, 3, NOG, 120] bf16: 3-tap conv bands, strided-row packing.
    Sub-block s, slot oi*30+m -> out channel og*4+oi, block row 3m+s."""
    wsum = conv_w.sum(axis=1)[:, :, 0].astype(np.float64)  # [O, 3]
    amc = np.zeros((M1, 3, NOG, M2), np.float32)
    for s in range(3):
        for og in range(NOG):
            for oi in range(OSUB):
                o = og * OSUB + oi
                for m in range(NS):
                    for j in range(3):
                        amc[3 * m + s + j, s, og, oi * NS + m] = wsum[o, j]
    return amc.astype(NPBF)


def _make_shard(xt_b, h0):
    """xt_b: [H, ROW] bf16 one batch (h-major rows). [370, ROW] zero-pad."""
    sh = np.zeros((SHR, ROW), NPBF)
    lo, hi = h0 - HALO, h0 + HS + HALO
    slo, shi = max(lo, 0), min(hi, H)
    sh[slo - lo:shi - lo] = xt_b[slo:shi]
    return sh


def make_in_maps(x, conv_w):
    x = np.ascontiguousarray(np.asarray(x, dtype=np.float32))
    conv_w = np.asarray(conv_w, dtype=np.float32)
    assert x.shape == (B, C, H, W) and conv_w.shape == (O, C, 3, 1)
    xt = np.ascontiguousarray(x.transpose(0, 2, 1, 3)).astype(NPBF)
    xt = xt.reshape(B, H, ROW)
    amc = _make_amc(conv_w)
    bands = [_make_bands(0), _make_bands(1)]
    in_maps = []
    for i in range(N_CORES):
        b, half = i // 2, i % 2
        in_maps.append({
            "xs": _make_shard(xt[b], half * HS),
            "bands": bands[half],
            "amc": amc,
        })
    return in_maps


def assemble_out(results):
    out = np.empty((B, O, H, W), np.float32)
    for i in range(N_CORES):
        b, half = i // 2, i % 2
        v = np.asarray(results[i]["out"]).astype(np.float32)
        ov = np.empty((O, HS, W), np.float32)
        for blk in range(NBLK):
            for s in range(3):
                w = v[blk, 0:M2, s, :, :]             # [4*30, NOG, W]
                w = w.reshape(OSUB, NS, NOG, W)
                w = w.transpose(2, 0, 1, 3).reshape(O, NS, W)
                ov[:, blk * BLK + s:blk * BLK + 90:3, :] = w
        out[b, :, half * HS:(half + 1) * HS, :] = ov
    return out


def kernel(x, conv_w):
    nc = bacc.Bacc("TRN2", target_bir_lowering=False, debug=False,
                   num_devices=N_CORES)
    _build(nc)
    nc.compile()
    res = run_bass_kernel_spmd(nc, make_in_maps(x, conv_w),
                               list(range(N_CORES)), trace=False)
    return assemble_out(res.results)
